# revision 1
# baseline (speedup 1.0000x reference)
"""GPT-2 small (12L, 12H, d768, S1024, B2, V50257) forward pass on 8 Trainium2 NeuronCores.

Sharding: 2-way data-parallel over batch x 4-way vocab shard of the unembed.
Cores 0-3 run batch item 0, cores 4-7 batch item 1 (trunk replicated within
each group of 4); each core computes logits for a 12800-wide vocab slice.
No collectives (measured ~58ms per 6.3MB AllReduce on this fabric -- far too
slow vs ~3ms of total compute).

Layout: activations are feature-major [D, S] so every linear layer contracts
over the partition dim with zero transposes.  LayerNorm statistics are
computed with ones-vector matmuls on the PE; broadcasts back over partitions
use K=1 ones-matmuls.  Attention processes head pairs so that Q and K slices
share a base partition (a matmul requirement).  Softmax skips the max
subtraction (scores are bounded ~2.4 for this model).  Causal structure
skips fully-masked score blocks; diagonal blocks are masked by multiplying
with 4 precomputed mask tiles.

Precision: f32r (tf32-class) matmuls for weights/layernorm, bf16 for
attention internals (Q/K/P/V, z, W_O).  Residual stream f32r.
"""
import os
import sys

for _p in ("/opt/trn_rl_repo", "/root/.axon_site/_ro/trn_rl_repo"):
    if os.path.isdir(_p) and _p not in sys.path:
        sys.path.append(_p)

import numpy as np
import ml_dtypes

import concourse.bass as bass
import concourse.mybir as mybir
import concourse.tile as tile
from concourse import bacc
from concourse import bass_utils

AF = mybir.ActivationFunctionType
ALU = mybir.AluOpType
F32 = mybir.dt.float32
F32R = mybir.dt.float32r
BF16 = mybir.dt.bfloat16
npbf16 = ml_dtypes.bfloat16

L, H, D, DH, DM, V, NCTX = 12, 12, 768, 64, 3072, 50257, 1024
B, S = 2, 1024
EPS = 1e-5
N_CORES = 8
KD = D // 128            # 6 feature k-tiles
KDM = DM // 128          # 24 dm tiles
NQT = S // 128           # 8 sequence 128-tiles
VSHARD = 12800           # padded per-core vocab shard (25 x 512)
NVC = VSHARD // 512

N_LAYERS = int(os.environ.get("BASS_GPT2_LAYERS", str(L)))
DO_UNEMBED = os.environ.get("BASS_GPT2_UNEMBED", "1") == "1"


def _emit_ln(nc, p_ps, p_x2, p_vec, onesk_t, onescol_t, eps_t, src, wt, bt, dst):
    """dst = LN(src over the feature/partition dim) * w + b (feature-major tiles)."""
    for qc in range(2):
        cs = slice(qc * 512, qc * 512 + 512)
        ps_s = p_ps.tile([1, 512], F32, tag="ps", name="ps_s")
        ps_s2 = p_ps.tile([1, 512], F32, tag="ps", name="ps_s2")
        for k in range(KD):
            x2t = p_x2.tile([128, 512], F32R, tag="x2", name="x2t")
            nc.scalar.square(x2t[:], src[k][:, cs])
            nc.tensor.matmul(ps_s[:], onesk_t[:], src[k][:, cs],
                             start=(k == 0), stop=(k == KD - 1))
            nc.tensor.matmul(ps_s2[:], onesk_t[:], x2t[:],
                             start=(k == 0), stop=(k == KD - 1))
        mu = p_vec.tile([1, 512], F32, tag="vec", name="mu")
        nc.vector.tensor_scalar_mul(mu[:], ps_s[:], 1.0 / D)
        mu2 = p_vec.tile([1, 512], F32, tag="vec", name="mu2")
        nc.vector.tensor_mul(mu2[:], mu[:], mu[:])
        # mu2 <- ex2 - mu^2  (in place), then sqrt(var+eps) in place
        nc.vector.scalar_tensor_tensor(mu2[:], ps_s2[:], 1.0 / D, mu2[:],
                                       op0=ALU.mult, op1=ALU.subtract)
        nc.scalar.activation(mu2[:], mu2[:], AF.Sqrt, bias=eps_t[0:1, 0:1], scale=1.0)
        rstd = p_vec.tile([1, 512], F32R, tag="vec", name="rstd")
        with nc.allow_low_precision(reason="ln rstd"):
            nc.vector.reciprocal(rstd[:], mu2[:])
        mrstd = p_vec.tile([1, 512], F32R, tag="vec", name="mrstd")
        nc.vector.tensor_mul(mrstd[:], mu[:], rstd[:])
        ps_rb = p_ps.tile([128, 512], F32, tag="ps", name="ps_rb")
        nc.tensor.matmul(ps_rb[:], onescol_t[0:1, :], rstd[:], start=True, stop=True)
        ps_mb = p_ps.tile([128, 512], F32, tag="ps", name="ps_mb")
        nc.tensor.matmul(ps_mb[:], onescol_t[0:1, :], mrstd[:], start=True, stop=True)
        for k in range(KD):
            nc.vector.tensor_mul(dst[k][:, cs], src[k][:, cs], ps_rb[:])
            nc.vector.tensor_sub(dst[k][:, cs], dst[k][:, cs], ps_mb[:])
            nc.scalar.activation(dst[k][:, cs], dst[k][:, cs], AF.Identity,
                                 bias=bt[:, k:k + 1], scale=wt[:, k:k + 1])


def build_nc(has_bv_bias: bool):
    nc = bacc.Bacc("TRN2", target_bir_lowering=False, debug=False, num_devices=N_CORES)

    def din(name, shape, dt):
        return nc.dram_tensor(name, list(shape), dt, kind="ExternalInput").ap()

    x0 = din("x0", [D, S], F32R)
    wq2 = din("wq2", [N_LAYERS, 6, KD, 128, 128], F32R)
    wk2 = din("wk2", [N_LAYERS, 6, KD, 128, 128], F32R)
    wv = din("wv", [N_LAYERS, KD, 128, H * DH], F32R)
    wo = din("wo", [N_LAYERS, H, DH, D], BF16)
    win = din("win", [N_LAYERS, KDM, KD, 128, 128], F32R)
    wout = din("wout", [N_LAYERS, KDM, 128, D], F32R)
    ln1w = din("ln1w", [N_LAYERS, 128, KD], F32)
    ln1b = din("ln1b", [N_LAYERS, 128, KD], F32)
    ln2w = din("ln2w", [N_LAYERS, 128, KD], F32)
    ln2b = din("ln2b", [N_LAYERS, 128, KD], F32)
    bqq = din("bqq", [N_LAYERS, 6, 128, 1], F32)
    bkk = din("bkk", [N_LAYERS, 6, 128, 1], F32)
    bvz = din("bvz", [N_LAYERS, 64, H], F32)
    bo = din("bo", [N_LAYERS, 128, KD], F32)
    bin_ = din("bin", [N_LAYERS, 128, KDM], F32)
    bout = din("bout", [N_LAYERS, 128, KD], F32)
    lnfw = din("lnfw", [128, KD], F32)
    lnfb = din("lnfb", [128, KD], F32)
    masks = din("masks", [128, 4 * 512], BF16)
    onescol = din("onescol", [65, 128], F32R)
    onesk = din("onesk", [128, 1], F32R)
    epsc = din("epsc", [1, 1], F32)
    if DO_UNEMBED:
        wu = din("wu", [KD, 128, VSHARD], F32R)
        out = nc.dram_tensor("out", [S, VSHARD], F32, kind="ExternalOutput").ap()
    else:
        out = nc.dram_tensor("out", [D, S], F32, kind="ExternalOutput").ap()

    from contextlib import ExitStack
    with tile.TileContext(nc) as tc:
        with ExitStack() as ctx:
            pools = {}
            for nm, bufs, space in (
                ("resid", 1, "SBUF"), ("vsm", 1, "SBUF"), ("const", 1, "SBUF"),
                ("lnp", 2, "SBUF"), ("xln", 6, "SBUF"), ("x2", 2, "SBUF"),
                ("vec", 5, "SBUF"), ("qkt", 3, "SBUF"), ("pt", 9, "SBUF"),
                ("zt", 12, "SBUF"), ("inv", 3, "SBUF"), ("bc", 3, "SBUF"),
                ("h", 2, "SBUF"), ("wqk", 3, "SBUF"), ("wv", 7, "SBUF"),
                ("wo", 2, "SBUF"), ("win", 2, "SBUF"), ("wout", 2, "SBUF"),
                ("wu", 6, "SBUF"), ("osb", 2, "SBUF"), ("ps", 8, "PSUM"),
            ):
                pools[nm] = ctx.enter_context(
                    tc.tile_pool(name=nm, bufs=bufs, space=space))
            p_resid, p_vsm, p_const, p_lnp, p_xln, p_x2, p_vec, p_qkt, p_pt, \
                p_zt, p_inv, p_bc, p_h, p_wqk, p_wv, p_wo, p_win, p_wout, \
                p_wu, p_osb, p_ps = (pools[n] for n in (
                    "resid", "vsm", "const", "lnp", "xln", "x2", "vec", "qkt",
                    "pt", "zt", "inv", "bc", "h", "wqk", "wv", "wo", "win",
                    "wout", "wu", "osb", "ps"))

            # ---- persistent state ----
            resid = []
            for k in range(KD):
                rt = p_resid.tile([128, S], F32R, tag=f"resid{k}", name=f"resid{k}")
                nc.sync.dma_start(rt[:], x0[k * 128:(k + 1) * 128, :])
                resid.append(rt)
            vsm = []
            for st in range(NQT):
                vt = p_vsm.tile([128, H * 65], BF16, tag=f"vsm{st}", name=f"vsm{st}")
                ones_ap = vt.rearrange("p (h c) -> p h c", c=65)[:, :, 64:65]
                nc.vector.memset(ones_ap, 1.0)
                vsm.append(vt)
            mask_t = p_const.tile([128, 4 * 512], BF16, tag="masks", name="mask_t")
            nc.sync.dma_start(mask_t[:], masks[:])
            onescol_t = p_const.tile([65, 128], F32R, tag="onescol", name="onescol_t")
            nc.sync.dma_start(onescol_t[:], onescol[:])
            onesk_t = p_const.tile([128, 1], F32R, tag="onesk", name="onesk_t")
            nc.sync.dma_start(onesk_t[:], onesk[:])
            eps_t = p_const.tile([1, 1], F32, tag="epsc", name="eps_t")
            nc.sync.dma_start(eps_t[:], epsc[:])
            lnfw_t = p_const.tile([128, KD], F32, tag="lnfw", name="lnfw_t")
            nc.sync.dma_start(lnfw_t[:], lnfw[:])
            lnfb_t = p_const.tile([128, KD], F32, tag="lnfb", name="lnfb_t")
            nc.sync.dma_start(lnfb_t[:], lnfb[:])

            def ln_params(dram_w, dram_b, l, tw, tb):
                wt = p_lnp.tile([128, KD], F32, tag=tw, name=tw)
                nc.sync.dma_start(wt[:], dram_w[l])
                bt = p_lnp.tile([128, KD], F32, tag=tb, name=tb)
                nc.sync.dma_start(bt[:], dram_b[l])
                return wt, bt

            for l in range(N_LAYERS):
                # ------- LN1 -------
                w1t, b1t = ln_params(ln1w, ln1b, l, "lnw", "lnb")
                xln = [p_xln.tile([128, S], F32R, tag="xln", name=f"xln{k}") for k in range(KD)]
                _emit_ln(nc, p_ps, p_x2, p_vec, onesk_t, onescol_t, eps_t, resid, w1t, b1t, xln)

                # ------- V projection (sequence-major, all heads) -------
                wv_tiles = []
                for ch, (c0, cw) in enumerate(((0, 512), (512, 256))):
                    for k in range(KD):
                        wv_t = p_wv.tile([128, 512], F32R, tag="wv", name="wv_t")
                        nc.sync.dma_start(wv_t[:, 0:cw], wv[l, k, :, c0:c0 + cw])
                        wv_tiles.append(wv_t)
                    for st in range(NQT):
                        ps_v = p_ps.tile([128, 512], F32, tag="ps", name="ps_v")
                        for k in range(KD):
                            nc.tensor.matmul(ps_v[:, 0:cw],
                                             xln[k][:, st * 128:(st + 1) * 128],
                                             wv_tiles[ch * KD + k][:, 0:cw],
                                             start=(k == 0), stop=(k == KD - 1))
                        nh = cw // DH
                        h0 = c0 // DH
                        dst = vsm[st].rearrange("p (h c) -> p h c", c=65)[:, h0:h0 + nh, 0:64]
                        src_ap = ps_v[:, 0:cw].rearrange("p (h e) -> p h e", e=64)
                        nc.scalar.copy(dst, src_ap)

                # ------- attention (head pairs) -------
                all_z = []
                for hp in range(6):
                    wq_t = p_wqk.tile([128, KD * 128], F32R, tag="wqk", name="wq_t")
                    nc.sync.dma_start(wq_t.rearrange("p (k m) -> p k m", k=KD), wq2[l, hp].rearrange("k p m -> p k m"))
                    wk_t = p_wqk.tile([128, KD * 128], F32R, tag="wqk", name="wk_t")
                    nc.sync.dma_start(wk_t.rearrange("p (k m) -> p k m", k=KD), wk2[l, hp].rearrange("k p m -> p k m"))
                    bq_t = p_lnp.tile([128, 1], F32, tag="bq", name="bq_t")
                    nc.sync.dma_start(bq_t[:], bqq[l, hp])
                    bk_t = p_lnp.tile([128, 1], F32, tag="bk", name="bk_t")
                    nc.sync.dma_start(bk_t[:], bkk[l, hp])
                    qq = p_qkt.tile([128, S], BF16, tag="qkt", name="qq")
                    kk = p_qkt.tile([128, S], BF16, tag="qkt", name="kk")
                    for qc in range(2):
                        cs = slice(qc * 512, qc * 512 + 512)
                        ps_q = p_ps.tile([128, 512], F32, tag="ps", name="ps_q")
                        ps_k = p_ps.tile([128, 512], F32, tag="ps", name="ps_k")
                        for k in range(KD):
                            nc.tensor.matmul(ps_q[:], wq_t[:, k * 128:(k + 1) * 128],
                                             xln[k][:, cs], start=(k == 0), stop=(k == KD - 1))
                        for k in range(KD):
                            nc.tensor.matmul(ps_k[:], wk_t[:, k * 128:(k + 1) * 128],
                                             xln[k][:, cs], start=(k == 0), stop=(k == KD - 1))
                        nc.scalar.activation(qq[:, cs], ps_q[:], AF.Identity,
                                             bias=bq_t[:, 0:1], scale=1.0)
                        nc.scalar.activation(kk[:, cs], ps_k[:], AF.Identity,
                                             bias=bk_t[:, 0:1], scale=1.0)
                    for j in range(2):
                        h = 2 * hp + j
                        e0 = 64 * j
                        z_t = p_zt.tile([64, S], BF16, tag="zt", name=f"z{h}")
                        for qc in range(2):
                            cs = slice(qc * 512, qc * 512 + 512)
                            nkt = 4 if qc == 0 else 8
                            pts = []
                            for kt in range(nkt):
                                ps_sc = p_ps.tile([128, 512], F32, tag="ps", name="ps_sc")
                                nc.tensor.matmul(ps_sc[:],
                                                 kk[e0:e0 + 64, kt * 128:(kt + 1) * 128],
                                                 qq[e0:e0 + 64, cs], start=True, stop=True)
                                pt = p_pt.tile([128, 512], BF16, tag="pt", name="pt")
                                nc.scalar.activation(pt[:], ps_sc[:], AF.Exp, scale=0.125)
                                r = kt * 128 - qc * 512
                                if 0 <= r <= 384:
                                    mc = (r // 128) * 512
                                    nc.vector.tensor_mul(pt[:], pt[:], mask_t[:, mc:mc + 512])
                                pts.append(pt)
                            ps_z = p_ps.tile([65, 512], F32, tag="ps", name="ps_z")
                            for kt in range(nkt):
                                nc.tensor.matmul(ps_z[:], vsm[kt][:, h * 65:(h + 1) * 65],
                                                 pts[kt][:], start=(kt == 0), stop=(kt == nkt - 1))
                            inv_t = p_inv.tile([65, 512], F32R, tag="inv", name="inv_t")
                            with nc.allow_low_precision(reason="softmax inv"):
                                nc.vector.reciprocal(inv_t[64:65, :], ps_z[64:65, :])
                            ps_bc = p_ps.tile([64, 512], F32, tag="ps", name="ps_bc")
                            nc.tensor.matmul(ps_bc[:], onescol_t[64:65, 0:64],
                                             inv_t[64:65, :], start=True, stop=True)
                            bc_sb = p_bc.tile([64, 512], F32, tag="bc", name="bc_sb")
                            nc.scalar.copy(bc_sb[:], ps_bc[:])
                            nc.vector.tensor_mul(z_t[:, cs], ps_z[0:64, :], bc_sb[:])
                            if has_bv_bias:
                                bv_t = p_lnp.tile([64, 1], F32, tag="bv", name="bv_t")
                                nc.sync.dma_start(bv_t[:], bvz[l, :, h:h + 1])
                                nc.vector.tensor_scalar_add(z_t[:, cs], z_t[:, cs], bv_t[:, 0:1])
                        all_z.append(z_t)

                # ------- O projection + residual -------
                bo_t = p_lnp.tile([128, KD], F32, tag="bo", name="bo_t")
                nc.sync.dma_start(bo_t[:], bo[l])
                for qc in range(2):
                    cs = slice(qc * 512, qc * 512 + 512)
                    ps_os = [p_ps.tile([128, 512], F32, tag="ps", name=f"ps_o{d}")
                             for d in range(KD)]
                    for h in range(H):
                        wo_t = p_wo.tile([DH, D], BF16, tag="wo", name="wo_t")
                        nc.sync.dma_start(wo_t[:], wo[l, h])
                        for d in range(KD):
                            nc.tensor.matmul(ps_os[d][:], wo_t[:, d * 128:(d + 1) * 128],
                                             all_z[h][:, cs], start=(h == 0), stop=(h == H - 1))
                    for d in range(KD):
                        nc.vector.scalar_tensor_tensor(resid[d][:, cs], ps_os[d][:],
                                                       bo_t[:, d:d + 1], resid[d][:, cs],
                                                       op0=ALU.add, op1=ALU.add)

                # ------- LN2 -------
                w2t, b2t = ln_params(ln2w, ln2b, l, "lnw", "lnb")
                yln = [p_xln.tile([128, S], F32R, tag="xln", name=f"yln{k}") for k in range(KD)]
                _emit_ln(nc, p_ps, p_x2, p_vec, onesk_t, onescol_t, eps_t, resid, w2t, b2t, yln)

                # ------- MLP (two q-chunk passes) -------
                bin_t = p_lnp.tile([128, KDM], F32, tag="bin", name="bin_t")
                nc.sync.dma_start(bin_t[:], bin_[l])
                bout_t = p_lnp.tile([128, KD], F32, tag="bout", name="bout_t")
                nc.sync.dma_start(bout_t[:], bout[l])
                for qc in range(2):
                    cs = slice(qc * 512, qc * 512 + 512)
                    ps_mo = [p_ps.tile([128, 512], F32, tag="ps", name=f"ps_mo{d}")
                             for d in range(KD)]
                    for dm in range(KDM):
                        win_t = p_win.tile([128, KD * 128], F32R, tag="win", name="win_t")
                        nc.sync.dma_start(win_t.rearrange("p (k m) -> p k m", k=KD), win[l, dm].rearrange("k p m -> p k m"))
                        wout_t = p_wout.tile([128, D], F32R, tag="wout", name="wout_t")
                        nc.sync.dma_start(wout_t[:], wout[l, dm])
                        ps_h = p_ps.tile([128, 512], F32, tag="ps", name="ps_h")
                        for k in range(KD):
                            nc.tensor.matmul(ps_h[:], win_t[:, k * 128:(k + 1) * 128],
                                             yln[k][:, cs], start=(k == 0), stop=(k == KD - 1))
                        h_t = p_h.tile([128, 512], F32R, tag="h", name="h_t")
                        nc.scalar.activation(h_t[:], ps_h[:], AF.Gelu_apprx_tanh,
                                             bias=bin_t[:, dm:dm + 1], scale=1.0)
                        for d in range(KD):
                            nc.tensor.matmul(ps_mo[d][:], wout_t[:, d * 128:(d + 1) * 128],
                                             h_t[:], start=(dm == 0), stop=(dm == KDM - 1))
                    for d in range(KD):
                        nc.vector.scalar_tensor_tensor(resid[d][:, cs], ps_mo[d][:],
                                                       bout_t[:, d:d + 1], resid[d][:, cs],
                                                       op0=ALU.add, op1=ALU.add)

            # ------- final LN + unembed -------
            xf = [p_xln.tile([128, S], F32R, tag="xln", name=f"xf{k}") for k in range(KD)]
            _emit_ln(nc, p_ps, p_x2, p_vec, onesk_t, onescol_t, eps_t, resid, lnfw_t, lnfb_t, xf)
            if DO_UNEMBED:
                for v in range(NVC):
                    wu_ts = []
                    for k in range(KD):
                        wu_t = p_wu.tile([128, 512], F32R, tag="wu", name="wu_t")
                        nc.sync.dma_start(wu_t[:], wu[k, :, v * 512:(v + 1) * 512])
                        wu_ts.append(wu_t)
                    for q in range(NQT):
                        ps_u = p_ps.tile([128, 512], F32, tag="ps", name="ps_u")
                        for k in range(KD):
                            nc.tensor.matmul(ps_u[:], xf[k][:, q * 128:(q + 1) * 128],
                                             wu_ts[k][:], start=(k == 0), stop=(k == KD - 1))
                        o_sb = p_osb.tile([128, 512], F32, tag="osb", name="o_sb")
                        if (v * NQT + q) % 2 == 0:
                            nc.vector.tensor_copy(o_sb[:], ps_u[:])
                        else:
                            nc.scalar.copy(o_sb[:], ps_u[:])
                        nc.sync.dma_start(out[q * 128:(q + 1) * 128, v * 512:(v + 1) * 512],
                                          o_sb[:])
            else:
                for k in range(KD):
                    for qc in range(2):
                        cs = slice(qc * 512, qc * 512 + 512)
                        o_sb = p_osb.tile([128, 512], F32, tag="osb", name="o_sb")
                        nc.vector.tensor_copy(o_sb[:], xf[k][:, cs])
                        nc.sync.dma_start(out[k * 128:(k + 1) * 128, cs], o_sb[:])

    nc.compile()
    return nc


# ------------------------- host side -------------------------

_BUILD_CACHE = {}
_PACK_CACHE = {}


def _get_nc(has_bv_bias):
    key = (N_LAYERS, DO_UNEMBED, has_bv_bias)
    if key not in _BUILD_CACHE:
        _BUILD_CACHE[key] = build_nc(has_bv_bias)
    return _BUILD_CACHE[key]


def _pack(inputs):
    key = (id(inputs.get("W_in")), id(inputs.get("W_U")))
    if _PACK_CACHE.get("key") == key:
        return _PACK_CACHE["packed"]
    f32 = np.float32
    W_Q = np.asarray(inputs["W_Q"], f32)[:N_LAYERS]
    W_K = np.asarray(inputs["W_K"], f32)[:N_LAYERS]
    W_V = np.asarray(inputs["W_V"], f32)[:N_LAYERS]
    W_O = np.asarray(inputs["W_O"], f32)[:N_LAYERS]
    W_in = np.asarray(inputs["W_in"], f32)[:N_LAYERS]
    W_out = np.asarray(inputs["W_out"], f32)[:N_LAYERS]
    NL = N_LAYERS

    def pack_qk(w):
        w = w.reshape(NL, H, KD, 128, DH)                      # l h k p e
        w = w.transpose(0, 2, 3, 1, 4).reshape(NL, KD, 128, 6, 2 * DH)
        return np.ascontiguousarray(w.transpose(0, 3, 1, 2, 4))  # l hp k p m

    wq2 = pack_qk(W_Q)
    wk2 = pack_qk(W_K)
    wv = np.ascontiguousarray(
        W_V.transpose(0, 2, 1, 3).reshape(NL, KD, 128, H * DH))
    wo = np.ascontiguousarray(W_O).astype(npbf16)
    win = np.ascontiguousarray(
        W_in.reshape(NL, KD, 128, KDM, 128).transpose(0, 3, 1, 2, 4))
    wout = np.ascontiguousarray(W_out.reshape(NL, KDM, 128, D))

    def pack_ln(x):
        x = np.asarray(x, f32)[:NL]
        return np.ascontiguousarray(x.reshape(NL, KD, 128).transpose(0, 2, 1))

    b_Q = np.asarray(inputs["b_Q"], f32)[:NL]
    b_K = np.asarray(inputs["b_K"], f32)[:NL]
    b_V = np.asarray(inputs["b_V"], f32)[:NL]

    m = np.zeros((128, 4, 512), npbf16)
    dk = np.arange(128)[:, None]
    dq = np.arange(512)[None, :]
    for ri, r in enumerate((0, 128, 256, 384)):
        m[:, ri, :] = ((r + dk) <= dq).astype(npbf16)

    W_U = np.asarray(inputs["W_U"], f32)
    wu_pad = np.zeros((D, 4 * VSHARD), f32)
    wu_pad[:, :V] = W_U
    wu_shards = [np.ascontiguousarray(
        wu_pad[:, c * VSHARD:(c + 1) * VSHARD].reshape(KD, 128, VSHARD)) for c in range(4)]

    packed = dict(
        wq2=wq2, wk2=wk2, wv=wv, wo=wo, win=win, wout=wout,
        ln1w=pack_ln(inputs["ln1_w"]), ln1b=pack_ln(inputs["ln1_b"]),
        ln2w=pack_ln(inputs["ln2_w"]), ln2b=pack_ln(inputs["ln2_b"]),
        bqq=np.ascontiguousarray(b_Q.reshape(NL, 6, 128, 1)),
        bkk=np.ascontiguousarray(b_K.reshape(NL, 6, 128, 1)),
        bvz=np.ascontiguousarray(b_V.transpose(0, 2, 1)),
        bo=pack_ln(inputs["b_O"]),
        bin=np.ascontiguousarray(
            np.asarray(inputs["b_in"], f32)[:NL].reshape(NL, KDM, 128).transpose(0, 2, 1)),
        bout=pack_ln(inputs["b_out"]),
        lnfw=np.ascontiguousarray(np.asarray(inputs["lnf_w"], f32).reshape(KD, 128).T),
        lnfb=np.ascontiguousarray(np.asarray(inputs["lnf_b"], f32).reshape(KD, 128).T),
        masks=np.ascontiguousarray(m.reshape(128, 4 * 512)),
        onescol=np.ones((65, 128), f32), onesk=np.ones((128, 1), f32),
        epsc=np.full((1, 1), EPS, f32),
        wu_shards=wu_shards, has_bv=bool(np.any(b_V != 0)),
    )
    _PACK_CACHE["key"] = key
    _PACK_CACHE["packed"] = packed
    return packed


def kernel(**inputs) -> np.ndarray:
    tokens = np.asarray(inputs["tokens"]).astype(np.int64)
    W_E = np.asarray(inputs["W_E"], np.float32)
    W_pos = np.asarray(inputs["W_pos"], np.float32)
    b_U = np.asarray(inputs["b_U"], np.float32)

    packed = _pack(inputs)
    nc = _get_nc(packed["has_bv"])

    resid0 = W_E[tokens] + W_pos[:S][None, :, :]
    x0 = [np.ascontiguousarray(resid0[b].T.astype(np.float32)) for b in range(B)]

    weight_names = ["wq2", "wk2", "wv", "wo", "win", "wout", "ln1w", "ln1b",
                    "ln2w", "ln2b", "bqq", "bkk", "bvz", "bo", "bin", "bout",
                    "lnfw", "lnfb", "masks", "onescol", "onesk", "epsc"]
    shared = {k: packed[k] for k in weight_names}
    in_maps = []
    for c in range(N_CORES):
        mcore = dict(shared)
        mcore["x0"] = x0[c // 4]
        if DO_UNEMBED:
            mcore["wu"] = packed["wu_shards"][c % 4]
        in_maps.append(mcore)

    res = bass_utils.run_bass_kernel_spmd(nc, in_maps, core_ids=list(range(N_CORES)))

    if DO_UNEMBED:
        logits = np.empty((B, S, V), np.float32)
        for b in range(B):
            full = np.concatenate([res.results[b * 4 + vs]["out"] for vs in range(4)], axis=1)
            logits[b] = full[:, :V]
        if np.any(b_U != 0):
            logits += b_U
        return logits
    else:
        return np.stack([res.results[0]["out"], res.results[4]["out"]])

